# revision 1
# baseline (speedup 1.0000x reference)
"""Trainium2 Bass kernel for multi-head self-attention (B=2, N=4096, C=512, H=8).

Sharding: 8 cores = 2 batches x 4 head-pairs. Core c handles batch c//4 and
heads {2*(c%4), 2*(c%4)+1}. Each core computes its two heads' attention over
all 4096 tokens and a partial output projection restricted to its heads' 128
channels; the host sums the 4 partials per batch (the tensor-parallel proj
all-reduce) and adds b_proj.

Device dataflow per core (fully fused, scores never touch DRAM):
  x[b] --PE transpose--> xT [512, 4096]
  kT = (wk^T @ x^T)        [128, 4096]   (rows 0-63 head0, 64-127 head1)
  qT likewise              [128, 4096]
  v  = x @ wv  (natural)   [4096, 130]   per 128-token tile: [Vh0 | 1 | Vh1 | 1]
  per 512-query chunk, per 128-key tile:
    S^T = kT_tile^T @ qT   (two concurrent K=64 row-packed matmuls)
    P^T = exp(SCALE * S^T) (ScalarE, FD=1024 straight out of PSUM)
    PV += [V|1]^T @ P^T    (PSUM accumulate; row 64 = softmax denominators)
  proj: out_qtile = (outT_h0^T @ wp_h0) * (1/sum_h0) + (outT_h1^T @ wp_h1) * (1/sum_h1)
  (denominators moved to partition layout with tiny K=1 matmuls)
"""

import os
import sys

if "/opt/trn_rl_repo" not in sys.path:
    sys.path.insert(0, "/opt/trn_rl_repo")

import numpy as np

import concourse.bass as bass
import concourse.mybir as mybir
import concourse.tile as tile
from concourse import bacc
from concourse.masks import make_identity

B, N, C, H = 2, 4096, 512, 8
D = C // H
SCALE = D**-0.5
F32 = mybir.dt.float32

MM_DT_NAME = os.environ.get("ATTN_MM_DT", "f32r")

_DT_MAP = {
    "f32": F32,
    "f32r": mybir.dt.float32r,
    "f16": mybir.dt.float16,
    "bf16": mybir.dt.bfloat16,
}


def build(tokens=N, mm_dt_name=MM_DT_NAME, timing=False):
    T = tokens
    n_xt = T // 128  # 128-token tiles (x tiles / key tiles / v tiles)
    n_s = T // 512  # 512-token slices for kT/qT
    n_qc = T // 512  # query chunks
    st_dt = _DT_MAP[mm_dt_name]  # storage dtype of matmul-operand tiles
    cast = st_dt != F32

    EXP = mybir.ActivationFunctionType.Exp
    MUL = mybir.AluOpType.mult
    ADD = mybir.AluOpType.add

    nc = bacc.Bacc(None)
    if timing:
        # timing variant: x/out stay device-local (Internal) so the per-
        # dispatch axon transfer is tiny; device-side work is identical.
        xb = nc.dram_tensor("xb", [T, C], F32)
        out = nc.dram_tensor("out", [T, C], F32)
        tok = nc.dram_tensor("tok", [128, C], F32, kind="ExternalOutput")
    else:
        xb = nc.dram_tensor("xb", [T, C], F32, kind="ExternalInput")
        out = nc.dram_tensor("out", [T, C], F32, kind="ExternalOutput")
        tok = None
    # weights arrive host-pre-transposed into SBUF layout [128, 512]:
    # w*_[p, kc*128 + j] = w[kc*128 + p, j]
    wq = nc.dram_tensor("wq", [128, 512], F32, kind="ExternalInput")
    wk = nc.dram_tensor("wk", [128, 512], F32, kind="ExternalInput")
    wv = nc.dram_tensor("wv", [128, 512], F32, kind="ExternalInput")
    wp = nc.dram_tensor("wp", [128, C], F32, kind="ExternalInput")

    with tile.TileContext(nc) as tc:
        with tc.tile_pool(name="persist", bufs=1) as pp:
            ident = pp.tile([128, 128], F32, tag="ident")
            make_identity(nc, ident[:])
            ones_col = pp.tile([128, 1], F32, tag="ones_col")
            nc.gpsimd.memset(ones_col[:], 1.0)
            if cast:
                ident_st = pp.tile([128, 128], st_dt, tag="ident_st")
                nc.vector.tensor_copy(ident_st[:], ident[:])
            else:
                ident_st = ident

            # --- weights -> SBUF (already in [128, 512] layout from the host)
            # always route via scratch+copy so matmul weight deps are DVE-only
            # (and the copy performs the required f32r rounding)
            w_sbs = {}
            for wname, wdram in (("wq", wq), ("wk", wk), ("wv", wv), ("wp", wp)):
                w_sb = pp.tile([128, 512], st_dt, tag=f"{wname}_sb", name=f"{wname}_sb")
                w_scr = pp.tile(
                    [128, 512], F32, tag=f"{wname}_scr", name=f"{wname}_scr"
                )
                nc.sync.dma_start(out=w_scr[:], in_=wdram[:, :])
                nc.vector.tensor_copy(w_sb[:], w_scr[:])
                w_sbs[wname] = w_sb
            wq_sb, wk_sb, wv_sb, wp_sb = (
                w_sbs["wq"],
                w_sbs["wk"],
                w_sbs["wv"],
                w_sbs["wp"],
            )

            kT = [
                pp.tile([128, 512], st_dt, tag=f"kT{s}", name=f"kT{s}")
                for s in range(n_s)
            ]
            qT = [
                pp.tile([128, 512], st_dt, tag=f"qT{s}", name=f"qT{s}")
                for s in range(n_s)
            ]
            v = [
                pp.tile([128, 130], st_dt, tag=f"v{t}", name=f"v{t}")
                for t in range(n_xt)
            ]
            outT = [
                pp.tile([128, 512], st_dt, tag=f"outT{s}", name=f"outT{s}")
                for s in range(n_qc)
            ]

            def attn_step(qc, kt, pv0, pv1, psS, ptp, n_last):
                sc = psS.tile([128, 1024], F32, tag="sc", name="sc")
                kslc = kT[kt // 4][:, (kt % 4) * 128 : (kt % 4 + 1) * 128]
                nc.tensor.matmul(
                    sc[:, 0:512],
                    kslc[0:64, :],
                    qT[qc][0:64, :],
                    start=True,
                    stop=True,
                    tile_position=(0, 0),
                )
                nc.tensor.matmul(
                    sc[:, 512:1024],
                    kslc[64:128, :],
                    qT[qc][64:128, :],
                    start=True,
                    stop=True,
                    tile_position=(64, 0),
                )
                pt = ptp.tile([128, 1024], st_dt, tag="pt", name="pt")
                nc.scalar.activation(pt[:], sc[:], EXP, scale=SCALE)
                nc.tensor.matmul(
                    pv0[:],
                    v[kt][:, 0:65],
                    pt[:, 0:512],
                    start=(kt == 0),
                    stop=(kt == n_last),
                )
                nc.tensor.matmul(
                    pv1[:],
                    v[kt][:, 65:130],
                    pt[:, 512:1024],
                    start=(kt == 0),
                    stop=(kt == n_last),
                )

            def drain(qc, pv0, pv1, smp):
                sums = smp.tile([1, 1024], F32, tag="sums", name="sums")
                nc.vector.tensor_copy(outT[qc][0:64, :], pv0[0:64, :])
                nc.vector.tensor_copy(sums[0:1, 0:512], pv0[64:65, :])
                nc.vector.tensor_copy(outT[qc][64:128, :], pv1[0:64, :])
                nc.vector.tensor_copy(sums[0:1, 512:1024], pv1[64:65, :])
                return sums

            def proj_qtile(qc, qs, sums, smp, osp, psT):
                    i = qc * 4 + qs
                    ta = psT.tile([128, 512], F32, tag="ta", name="ta")
                    tb = psT.tile([128, 512], F32, tag="tb", name="tb")
                    # denominators -> partition layout via K=1 fp32 matmuls
                    nc.tensor.matmul(
                        ta[:, 0:1],
                        sums[0:1, qs * 128 : (qs + 1) * 128],
                        ident[0:1, 0:1],
                        start=True,
                        stop=True,
                    )
                    nc.tensor.matmul(
                        ta[:, 1:2],
                        sums[0:1, 512 + qs * 128 : 512 + (qs + 1) * 128],
                        ident[0:1, 0:1],
                        start=True,
                        stop=True,
                    )
                    rc = smp.tile([128, 2], F32, tag="recip", name="rc")
                    nc.vector.reciprocal(rc[:], ta[:, 0:2])
                    oslc = outT[qc][:, qs * 128 : (qs + 1) * 128]
                    nc.tensor.matmul(
                        ta[:],
                        oslc[0:64, :],
                        wp_sb[0:64, :],
                        start=True,
                        stop=True,
                        tile_position=(0, 0),
                    )
                    nc.tensor.matmul(
                        tb[:],
                        oslc[64:128, :],
                        wp_sb[64:128, :],
                        start=True,
                        stop=True,
                        tile_position=(64, 0),
                    )
                    t0 = osp.tile([128, 512], F32, tag="t0", name="t0")
                    nc.vector.tensor_scalar_mul(t0[:], ta[:], rc[:, 0:1])
                    ob = osp.tile([128, 512], F32, tag="ob", name="ob")
                    nc.vector.scalar_tensor_tensor(
                        ob[:], tb[:], rc[:, 1:2], t0[:], op0=MUL, op1=ADD
                    )
                    nc.sync.dma_start(out=out[i * 128 : (i + 1) * 128, :], in_=ob[:])
                    if timing and i == 4 * n_qc - 1:
                        nc.sync.dma_start(out=tok[:, :], in_=ob[:])

            with tc.tile_pool(name="ptp", bufs=3) as ptp, tc.tile_pool(
                name="smp", bufs=2
            ) as smp, tc.tile_pool(name="osp", bufs=2) as osp, tc.tile_pool(
                name="psS", bufs=2, space="PSUM"
            ) as psS, tc.tile_pool(name="psV", bufs=1, space="PSUM") as psV:
                pv0_0 = psV.tile([65, 512], F32, tag="pv0", name="pv0")
                pv1_0 = psV.tile([65, 512], F32, tag="pv1", name="pv1")

                # ---- prologue: produce xT/kT/qT/v per 512-token group, with
                # qc=0's attention interleaved so ScalarE starts early
                with tc.tile_pool(name="xtp", bufs=1) as xtp, tc.tile_pool(
                    name="ldp", bufs=4
                ) as ldp, tc.tile_pool(name="psA", bufs=2, space="PSUM") as psA:
                    xTs = [
                        xtp.tile([128, 2048], st_dt, tag=f"xTs{s}", name=f"xTs{s}")
                        for s in range(n_s)
                    ]
                    for s in range(n_s):
                        for t in range(4 * s, 4 * s + 4):
                            j = t % 4
                            xs = ldp.tile([128, C], F32, tag="xload", name="xs")
                            nc.sync.dma_start(
                                out=xs[:], in_=xb[t * 128 : (t + 1) * 128, :]
                            )
                            wsl = psA.tile([128, 512], F32, tag="work", name="x_tr")
                            for kc in range(4):
                                nc.tensor.transpose(
                                    wsl[:, kc * 128 : (kc + 1) * 128],
                                    xs[:, kc * 128 : (kc + 1) * 128],
                                    ident[:],
                                )
                            # one strided copy scatters the 4 blocks into xTs[s]
                            dst = xTs[s].rearrange("p (c w) -> p c w", c=4)[
                                :, :, j * 128 : (j + 1) * 128
                            ]
                            nc.vector.tensor_copy(
                                dst, wsl.rearrange("p (c w) -> p c w", c=4)
                            )
                        for w_sb, dst in ((wk_sb, kT), (wq_sb, qT)):
                            ps = psA.tile([128, 512], F32, tag="work", name="ps_kv")
                            for kc in range(4):
                                nc.tensor.matmul(
                                    ps[:],
                                    w_sb[:, kc * 128 : (kc + 1) * 128],
                                    xTs[s][:, kc * 512 : (kc + 1) * 512],
                                    start=(kc == 0),
                                    stop=(kc == 3),
                                )
                            nc.vector.tensor_copy(dst[s][:], ps[:])
                        # vT for this group, then transpose to natural v tiles
                        vtp = psA.tile([128, 512], F32, tag="work", name="vt_ps")
                        for kc in range(4):
                            nc.tensor.matmul(
                                vtp[:],
                                wv_sb[:, kc * 128 : (kc + 1) * 128],
                                xTs[s][:, kc * 512 : (kc + 1) * 512],
                                start=(kc == 0),
                                stop=(kc == 3),
                            )
                        vts = ldp.tile([128, 512], st_dt, tag="vts", name="vts")
                        nc.vector.tensor_copy(vts[:], vtp[:])
                        for t in range(4 * s, 4 * s + 4):
                            j = t % 4
                            vps = psA.tile([128, 128], st_dt, tag="work", name="v_tr")
                            nc.tensor.transpose(
                                vps[:], vts[:, j * 128 : (j + 1) * 128], ident_st[:]
                            )
                            nc.vector.tensor_copy(v[t][:, 0:64], vps[:, 0:64])
                            nc.vector.tensor_copy(v[t][:, 65:129], vps[:, 64:128])
                            nc.vector.tensor_copy(v[t][:, 64:65], ones_col[:])
                            nc.vector.tensor_copy(v[t][:, 129:130], ones_col[:])
                        # qc=0 attention over this group's key tiles
                        for kt in range(4 * s, 4 * s + 4):
                            attn_step(0, kt, pv0_0, pv1_0, psS, ptp, n_xt - 1)

                # ---- steady state: remaining chunks; each chunk's
                # projection is interleaved into the NEXT chunk's kt loop so
                # its PE/DVE work hides under ScalarE's exp stream.
                with tc.tile_pool(name="psT", bufs=1, space="PSUM") as psT:
                    prev = drain(0, pv0_0, pv1_0, smp) if n_qc > 0 else None
                    prev_qc = 0
                    for qc in range(1, n_qc):
                        pv0 = psV.tile([65, 512], F32, tag="pv0", name="pv0")
                        pv1 = psV.tile([65, 512], F32, tag="pv1", name="pv1")
                        for kt in range(n_xt):
                            attn_step(qc, kt, pv0, pv1, psS, ptp, n_xt - 1)
                            if kt % 8 == 7 and kt // 8 < 3:
                                proj_qtile(prev_qc, kt // 8, prev, smp, osp, psT)
                        proj_qtile(prev_qc, 3, prev, smp, osp, psT)
                        prev = drain(qc, pv0, pv1, smp)
                        prev_qc = qc
                    for qs in range(4):
                        proj_qtile(prev_qc, qs, prev, smp, osp, psT)
    nc.compile()
    return nc


_CACHE = {}


def _get_nc(tokens=N, mm_dt_name=MM_DT_NAME):
    key = (tokens, mm_dt_name)
    if key not in _CACHE:
        _CACHE[key] = build(tokens, mm_dt_name)
    return _CACHE[key]


def _prep_w(w_slice):
    """[512, 128] -> [128, 512] with layout w_[p, kc*128 + j] = w[kc*128 + p, j]."""
    w = np.asarray(w_slice, dtype=np.float32)
    return np.ascontiguousarray(
        w.reshape(4, 128, 128).transpose(1, 0, 2).reshape(128, 512)
    )


def _shard_inputs(x, w_qkv, w_proj):
    in_maps = []
    for c in range(8):
        b, hp = divmod(c, 4)
        o = 128 * hp
        in_maps.append(
            {
                "xb": np.ascontiguousarray(x[b], dtype=np.float32),
                "wq": _prep_w(w_qkv[:, o : o + 128]),
                "wk": _prep_w(w_qkv[:, 512 + o : 512 + o + 128]),
                "wv": _prep_w(w_qkv[:, 1024 + o : 1024 + o + 128]),
                "wp": np.ascontiguousarray(w_proj[o : o + 128, :], dtype=np.float32),
            }
        )
    return in_maps


def run(x, w_qkv, w_proj, b_proj, trace=False, **kwargs):
    from concourse.bass_utils import run_bass_kernel_spmd

    nc = _get_nc()
    in_maps = _shard_inputs(
        np.asarray(x), np.asarray(w_qkv), np.asarray(w_proj)
    )
    br = run_bass_kernel_spmd(nc, in_maps, list(range(8)), trace=trace, **kwargs)
    parts = [np.asarray(br.results[c]["out"]) for c in range(8)]
    bp = np.asarray(b_proj)
    o0 = parts[0] + parts[1] + parts[2] + parts[3] + bp
    o1 = parts[4] + parts[5] + parts[6] + parts[7] + bp
    return np.stack([o0, o1]).astype(np.float32), br


def kernel(x, w_qkv, w_proj, b_proj):
    result, _ = run(x, w_qkv, w_proj, b_proj, trace=False)
    return result



# revision 18
# speedup vs baseline: 1.1607x; 1.1607x over previous
"""Trainium2 Bass kernel for multi-head self-attention (B=2, N=4096, C=512, H=8).

Sharding: 8 cores = 2 batches x 4 head-pairs. Core c handles batch c//4 and
heads {2*(c%4), 2*(c%4)+1}. Each core computes its two heads' attention over
all 4096 tokens and a partial output projection restricted to its heads' 128
channels; the host sums the 4 partials per batch (the tensor-parallel proj
all-reduce) and adds b_proj.

V1 dataflow (fp16 operands, fp32 PSUM accumulation, scores never in DRAM):
  xT arrives host-pre-transposed [512, 4096] fp16 -> SBUF (no PE transposes)
  kT/qT = (w^T @ xT)              [128, 4096]  (rows 0-63 head0, 64-127 head1)
  v natural = xT-block^T @ wv     [4096, 130]  per tile: [Vh0 | 1 | Vh1 | 1]
  per 512-query chunk, per 128-key tile:
    S^T = kslc^T @ qT   (two row-packed K=64 matmuls -> PSUM [128, 1024])
    P^T = exp(SCALE * S^T) fp16   (ScalarE, straight out of PSUM)
    PV += [V|1]^T @ P^T           (PSUM accumulate; row 64 = denominators)
  chunk epilogue: rc = 1/denoms (DVE approx), bcast rows via tiny matmul,
    outT = PV * rc (fused drain+scale, fp16), then ONE packed proj matmul
    per 128-query tile: out_qtile = outT_blk^T @ wp  (both heads in one K=128)
"""

import os
import sys

if "/opt/trn_rl_repo" not in sys.path:
    sys.path.insert(0, "/opt/trn_rl_repo")

import numpy as np

import concourse.mybir as mybir
import concourse.tile as tile
from concourse import bacc

B, N, C, H = 2, 4096, 512, 8
D = C // H
SCALE = D**-0.5
MM_DT_NAME = "f16"  # informational (test.py prints it)
F32 = mybir.dt.float32
F16 = mybir.dt.float16
MUL = mybir.AluOpType.mult
EXP = mybir.ActivationFunctionType.Exp


def build(tokens=N):
    T = tokens
    n_xt = T // 128  # key tiles
    n_s = T // 512  # token slices for kT/qT production
    n_qc = T // 512  # query chunks

    nc = bacc.Bacc(None)
    xt = nc.dram_tensor("xt", [C, T], F16, kind="ExternalInput")  # x[b].T
    out = nc.dram_tensor("out", [T, C], F32, kind="ExternalOutput")
    # w*_[p, kc*128 + j] = w[kc*128 + p, j]  (host-prepped, fp16)
    wq = nc.dram_tensor("wq", [128, 512], F16, kind="ExternalInput")
    wk = nc.dram_tensor("wk", [128, 512], F16, kind="ExternalInput")
    wv = nc.dram_tensor("wv", [128, 512], F16, kind="ExternalInput")
    wp = nc.dram_tensor("wp", [128, C], F16, kind="ExternalInput")  # natural

    with tile.TileContext(nc) as tc:
        with tc.tile_pool(name="persist", bufs=1) as pp:
            w_sbs = {}
            for wname, wdram in (("wq", wq), ("wk", wk), ("wv", wv), ("wp", wp)):
                w_sb = pp.tile([128, 512], F16, tag=f"{wname}_sb", name=f"{wname}_sb")
                nc.sync.dma_start(out=w_sb[:], in_=wdram[:, :])
                w_sbs[wname] = w_sb
            wq_sb, wk_sb, wv_sb, wp_sb = (
                w_sbs["wq"],
                w_sbs["wk"],
                w_sbs["wv"],
                w_sbs["wp"],
            )
            # ones row for broadcasting recip rows across 64 partitions
            ones1 = pp.tile([1, 64], F16, tag="ones1")
            nc.gpsimd.memset(ones1[:], 1.0)

            xTs = [
                pp.tile([128, T], F16, tag=f"xTs{kc}", name=f"xTs{kc}")
                for kc in range(4)
            ]
            kT = [
                pp.tile([128, 512], F16, tag=f"kT{s}", name=f"kT{s}")
                for s in range(n_s)
            ]
            qT = [
                pp.tile([128, 512], F16, tag=f"qT{s}", name=f"qT{s}")
                for s in range(n_s)
            ]
            v = [
                pp.tile([128, 130], F16, tag=f"v{t}", name=f"v{t}")
                for t in range(n_xt)
            ]

            def attn_S(qc, kt, psS, ptp):
                """Scores + exp for one (chunk, key-tile); returns pt tile."""
                sc = psS.tile([128, 1024], F32, tag="sc", name="sc")
                kslc = kT[kt // 4][:, (kt % 4) * 128 : (kt % 4 + 1) * 128]
                nc.tensor.matmul(
                    sc[:, 0:512],
                    kslc[0:64, :],
                    qT[qc][0:64, :],
                    start=True,
                    stop=True,
                    tile_position=(0, 0),
                )
                nc.tensor.matmul(
                    sc[:, 512:1024],
                    kslc[64:128, :],
                    qT[qc][64:128, :],
                    start=True,
                    stop=True,
                    tile_position=(64, 0),
                )
                pt = ptp.tile([128, 1024], F16, tag="pt", name="pt")
                nc.scalar.activation(pt[:], sc[:], EXP, scale=SCALE)
                return pt

            def attn_PV(kt, pt, pv0, pv1):
                nc.tensor.matmul(
                    pv0[:],
                    v[kt][:, 0:65],
                    pt[:, 0:512],
                    start=(kt == 0),
                    stop=(kt == n_xt - 1),
                )
                nc.tensor.matmul(
                    pv1[:],
                    v[kt][:, 65:130],
                    pt[:, 512:1024],
                    start=(kt == 0),
                    stop=(kt == n_xt - 1),
                )

            def epi(pv0, pv1, smp, otp, psB):
                """Chunk epilogue: recip of denominators, broadcast, fused
                drain+scale. Frees the pv banks; returns outT (fp16 [128,512],
                rows 0-63 head0 dims, 64-127 head1 dims, pre-normalized)."""
                dna = smp.tile([1, 512], F32, tag="dna", name="dna")
                dnb = smp.tile([1, 512], F32, tag="dnb", name="dnb")
                nc.vector.tensor_copy(dna[:], pv0[64:65, :])
                nc.vector.tensor_copy(dnb[:], pv1[64:65, :])
                rca = smp.tile([1, 512], F32, tag="rca", name="rca")
                rcb = smp.tile([1, 512], F32, tag="rcb", name="rcb")
                nc.vector.reciprocal_approx_fast(rca[:], dna[:])
                nc.vector.reciprocal_approx_fast(rcb[:], dnb[:])
                rha = smp.tile([1, 512], F16, tag="rha", name="rha")
                rhb = smp.tile([1, 512], F16, tag="rhb", name="rhb")
                nc.vector.tensor_copy(rha[:], rca[:])
                nc.vector.tensor_copy(rhb[:], rcb[:])
                bc = psB.tile([128, 512], F32, tag="bc", name="bc")
                nc.tensor.matmul(bc[0:64, :], ones1[:], rha[:], start=True, stop=True)
                nc.tensor.matmul(
                    bc[64:128, :],
                    ones1[:],
                    rhb[:],
                    start=True,
                    stop=True,
                    tile_position=(0, 64),
                )
                bc_sb = otp.tile([128, 512], F32, tag="bc_sb", name="bc_sb")
                nc.vector.tensor_copy(bc_sb[:], bc[:])
                outT = otp.tile([128, 512], F16, tag="outT", name="outT")
                nc.vector.tensor_tensor(
                    outT[0:64, :], pv0[0:64, :], bc_sb[0:64, :], op=MUL
                )
                nc.vector.tensor_tensor(
                    outT[64:128, :], pv1[0:64, :], bc_sb[64:128, :], op=MUL
                )
                return outT

            def proj_qtile(qc, qs, outT, psP, obp):
                i = qc * 4 + qs
                pj = psP.tile([128, 512], F32, tag="pj", name="pj")
                nc.tensor.matmul(
                    pj[:],
                    outT[:, qs * 128 : (qs + 1) * 128],
                    wp_sb[:],
                    start=True,
                    stop=True,
                )
                ob = obp.tile([128, 512], F32, tag="ob", name="ob")
                nc.vector.tensor_copy(ob[:], pj[:])
                nc.sync.dma_start(out=out[i * 128 : (i + 1) * 128, :], in_=ob[:])

            with tc.tile_pool(name="ptp", bufs=5) as ptp, tc.tile_pool(
                name="smp", bufs=2
            ) as smp, tc.tile_pool(name="otp", bufs=2) as otp, tc.tile_pool(
                name="obp", bufs=2
            ) as obp, tc.tile_pool(
                name="psS", bufs=2, space="PSUM"
            ) as psS, tc.tile_pool(name="psV", bufs=1, space="PSUM") as psV:
                pv0 = psV.tile([65, 512], F32, tag="pv0", name="pv0")
                pv1 = psV.tile([65, 512], F32, tag="pv1", name="pv1")

                # ---- prologue: per 512-token slice produce kT/qT/v, with
                # qc=0's attention interleaved so ScalarE starts early
                with tc.tile_pool(name="psA", bufs=2, space="PSUM") as psA:
                    for s in range(n_s):
                        sl = slice(s * 512, (s + 1) * 512)
                        for kc in range(4):
                            nc.sync.dma_start(
                                out=xTs[kc][:, sl],
                                in_=xt[kc * 128 : (kc + 1) * 128, sl],
                            )
                        for w_sb, dst in ((wk_sb, kT), (wq_sb, qT)):
                            ps = psA.tile([128, 512], F32, tag="work", name="ps_kq")
                            for kc in range(4):
                                nc.tensor.matmul(
                                    ps[:],
                                    w_sb[:, kc * 128 : (kc + 1) * 128],
                                    xTs[kc][:, sl],
                                    start=(kc == 0),
                                    stop=(kc == 3),
                                )
                            nc.vector.tensor_copy(dst[s][:], ps[:])
                        # v natural: per 128-token block, accumulate over kc
                        vn = psA.tile([128, 512], F32, tag="work", name="vn")
                        for j in range(4):
                            tb = slice((4 * s + j) * 128, (4 * s + j + 1) * 128)
                            for kc in range(4):
                                nc.tensor.matmul(
                                    vn[:, j * 128 : (j + 1) * 128],
                                    xTs[kc][:, tb],
                                    wv_sb[:, kc * 128 : (kc + 1) * 128],
                                    start=(kc == 0),
                                    stop=(kc == 3),
                                )
                        for j in range(4):
                            t = 4 * s + j
                            nc.vector.tensor_copy(
                                v[t][:, 0:64], vn[:, j * 128 : j * 128 + 64]
                            )
                            nc.vector.tensor_copy(
                                v[t][:, 65:129], vn[:, j * 128 + 64 : (j + 1) * 128]
                            )
                            nc.gpsimd.memset(v[t][:, 64:65], 1.0)
                            nc.gpsimd.memset(v[t][:, 129:130], 1.0)
                        for kt in range(4 * s, 4 * s + 4):
                            pt = attn_S(0, kt, psS, ptp)
                            attn_PV(kt, pt, pv0, pv1)

                # ---- steady state: chunks 1..n_qc-1. Chunk qc-1's epilogue is
                # interleaved into chunk qc's kt loop: the first DEFER steps
                # emit only S+exp (PE stays busy) while the DVE drains the
                # previous chunk's PV accumulators; PV matmuls for those steps
                # are emitted after the drain so the in-order PE never blocks.
                DEFER = 3
                with tc.tile_pool(name="psB", bufs=1, space="PSUM") as psB, (
                    tc.tile_pool(name="psP", bufs=1, space="PSUM")
                ) as psP:
                    prev_pv = (pv0, pv1)
                    prev_qc = 0
                    outT = None
                    for qc in range(1, n_qc):
                        pv0 = psV.tile([65, 512], F32, tag="pv0", name="pv0")
                        pv1 = psV.tile([65, 512], F32, tag="pv1", name="pv1")
                        pts = [attn_S(qc, kt, psS, ptp) for kt in range(DEFER)]
                        outT = epi(prev_pv[0], prev_pv[1], smp, otp, psB)
                        for kt in range(DEFER):
                            attn_PV(kt, pts[kt], pv0, pv1)
                        for kt in range(DEFER, n_xt):
                            pt = attn_S(qc, kt, psS, ptp)
                            attn_PV(kt, pt, pv0, pv1)
                            if kt % 8 == 7:
                                proj_qtile(prev_qc, kt // 8, outT, psP, obp)
                        prev_pv = (pv0, pv1)
                        prev_qc = qc
                    outT = epi(prev_pv[0], prev_pv[1], smp, otp, psB)
                    for qs in range(4):
                        proj_qtile(prev_qc, qs, outT, psP, obp)
    nc.compile()
    return nc


_CACHE = {}


def _get_nc(tokens=N):
    if tokens not in _CACHE:
        _CACHE[tokens] = build(tokens)
    return _CACHE[tokens]


def _prep_w(w_slice):
    """[512, 128] -> [128, 512] fp16 with w_[p, kc*128 + j] = w[kc*128 + p, j]."""
    w = np.asarray(w_slice, dtype=np.float32)
    return np.ascontiguousarray(
        w.reshape(4, 128, 128).transpose(1, 0, 2).reshape(128, 512).astype(np.float16)
    )


def _shard_inputs(x, w_qkv, w_proj):
    in_maps = []
    for c in range(8):
        b, hp = divmod(c, 4)
        o = 128 * hp
        in_maps.append(
            {
                "xt": np.ascontiguousarray(x[b].T.astype(np.float16)),
                "wq": _prep_w(w_qkv[:, o : o + 128]),
                "wk": _prep_w(w_qkv[:, 512 + o : 512 + o + 128]),
                "wv": _prep_w(w_qkv[:, 1024 + o : 1024 + o + 128]),
                "wp": np.ascontiguousarray(
                    w_proj[o : o + 128, :].astype(np.float16)
                ),
            }
        )
    return in_maps


def run(x, w_qkv, w_proj, b_proj, trace=False, **kwargs):
    from concourse.bass_utils import run_bass_kernel_spmd

    nc = _get_nc()
    in_maps = _shard_inputs(np.asarray(x), np.asarray(w_qkv), np.asarray(w_proj))
    br = run_bass_kernel_spmd(nc, in_maps, list(range(8)), trace=trace, **kwargs)
    parts = [np.asarray(br.results[c]["out"]) for c in range(8)]
    bp = np.asarray(b_proj)
    o0 = parts[0] + parts[1] + parts[2] + parts[3] + bp
    o1 = parts[4] + parts[5] + parts[6] + parts[7] + bp
    return np.stack([o0, o1]).astype(np.float32), br


def kernel(x, w_qkv, w_proj, b_proj):
    result, _ = run(x, w_qkv, w_proj, b_proj, trace=False)
    return result


# revision 20
# speedup vs baseline: 1.2439x; 1.0717x over previous
"""Trainium2 Bass kernel for multi-head self-attention (B=2, N=4096, C=512, H=8).

Sharding: 8 cores = 2 batches x 4 head-pairs. Core c handles batch c//4 and
heads {2*(c%4), 2*(c%4)+1}. Each core computes its two heads' attention over
all 4096 tokens and a partial output projection restricted to its heads' 128
channels; the host sums the 4 partials per batch (the tensor-parallel proj
all-reduce) and adds b_proj.

V1 dataflow (fp16 operands, fp32 PSUM accumulation, scores never in DRAM):
  xT arrives host-pre-transposed [512, 4096] fp16 -> SBUF (no PE transposes)
  kT/qT = (w^T @ xT)              [128, 4096]  (rows 0-63 head0, 64-127 head1)
  v natural = xT-block^T @ wv     [4096, 130]  per tile: [Vh0 | 1 | Vh1 | 1]
  per 512-query chunk, per 128-key tile:
    S^T = kslc^T @ qT   (two row-packed K=64 matmuls -> PSUM [128, 1024])
    P^T = exp(SCALE * S^T) fp16   (ScalarE, straight out of PSUM)
    PV += [V|1]^T @ P^T           (PSUM accumulate; row 64 = denominators)
  chunk epilogue: rc = 1/denoms (DVE approx), bcast rows via tiny matmul,
    outT = PV * rc (fused drain+scale, fp16), then ONE packed proj matmul
    per 128-query tile: out_qtile = outT_blk^T @ wp  (both heads in one K=128)
"""

import os
import sys

if "/opt/trn_rl_repo" not in sys.path:
    sys.path.insert(0, "/opt/trn_rl_repo")

import numpy as np

import concourse.mybir as mybir
import concourse.tile as tile
from concourse import bacc

B, N, C, H = 2, 4096, 512, 8
D = C // H
SCALE = D**-0.5
MM_DT_NAME = "f16"  # informational (test.py prints it)
F32 = mybir.dt.float32
F16 = mybir.dt.float16
MUL = mybir.AluOpType.mult
EXP = mybir.ActivationFunctionType.Exp


def build(tokens=N):
    T = tokens
    n_xt = T // 128  # key tiles
    n_s = T // 512  # token slices for kT/qT production
    n_qc = T // 512  # query chunks

    nc = bacc.Bacc(None)
    xt = nc.dram_tensor("xt", [C, T], F16, kind="ExternalInput")  # x[b].T
    out = nc.dram_tensor("out", [T, C], F32, kind="ExternalOutput")
    # w*_[p, kc*128 + j] = w[kc*128 + p, j]  (host-prepped, fp16)
    wq = nc.dram_tensor("wq", [128, 512], F16, kind="ExternalInput")
    wk = nc.dram_tensor("wk", [128, 512], F16, kind="ExternalInput")
    wv = nc.dram_tensor("wv", [128, 512], F16, kind="ExternalInput")
    wp = nc.dram_tensor("wp", [128, C], F16, kind="ExternalInput")  # natural

    with tile.TileContext(nc) as tc:
        with tc.tile_pool(name="persist", bufs=1) as pp:
            w_sbs = {}
            for wname, wdram in (("wq", wq), ("wk", wk), ("wv", wv), ("wp", wp)):
                w_sb = pp.tile([128, 512], F16, tag=f"{wname}_sb", name=f"{wname}_sb")
                nc.sync.dma_start(out=w_sb[:], in_=wdram[:, :])
                w_sbs[wname] = w_sb
            wq_sb, wk_sb, wv_sb, wp_sb = (
                w_sbs["wq"],
                w_sbs["wk"],
                w_sbs["wv"],
                w_sbs["wp"],
            )
            # ones row for broadcasting recip rows across 64 partitions
            ones1 = pp.tile([1, 64], F16, tag="ones1")
            nc.gpsimd.memset(ones1[:], 1.0)

            xTs = [
                pp.tile([128, T], F16, tag=f"xTs{kc}", name=f"xTs{kc}")
                for kc in range(4)
            ]
            kT = [
                pp.tile([128, 512], F16, tag=f"kT{s}", name=f"kT{s}")
                for s in range(n_s)
            ]
            qT = [
                pp.tile([128, 512], F16, tag=f"qT{s}", name=f"qT{s}")
                for s in range(n_s)
            ]
            v = [
                pp.tile([128, 130], F16, tag=f"v{t}", name=f"v{t}")
                for t in range(n_xt)
            ]

            def attn_S(qc, kt, psS, ptp):
                """Scores + exp for one (chunk, key-tile); returns pt tile."""
                sc = psS.tile([128, 1024], F32, tag="sc", name="sc")
                kslc = kT[kt // 4][:, (kt % 4) * 128 : (kt % 4 + 1) * 128]
                nc.tensor.matmul(
                    sc[:, 0:512],
                    kslc[0:64, :],
                    qT[qc][0:64, :],
                    start=True,
                    stop=True,
                    tile_position=(0, 0),
                )
                nc.tensor.matmul(
                    sc[:, 512:1024],
                    kslc[64:128, :],
                    qT[qc][64:128, :],
                    start=True,
                    stop=True,
                    tile_position=(64, 0),
                )
                pt = ptp.tile([128, 1024], F16, tag="pt", name="pt")
                nc.scalar.activation(pt[:], sc[:], EXP, scale=SCALE)
                return pt

            def attn_PV(kt, pt, pv0, pv1):
                nc.tensor.matmul(
                    pv0[:],
                    v[kt][:, 0:65],
                    pt[:, 0:512],
                    start=(kt == 0),
                    stop=(kt == n_xt - 1),
                )
                nc.tensor.matmul(
                    pv1[:],
                    v[kt][:, 65:130],
                    pt[:, 512:1024],
                    start=(kt == 0),
                    stop=(kt == n_xt - 1),
                )

            def epi_drain(pv0, pv1, smp, otp):
                """Fast PV-bank release: raw copies only (DVE, ~2.7us).
                Emitted right after the chunk's last PV so the next chunk's
                deferred S/exp steps hide it."""
                dna = smp.tile([1, 512], F32, tag="dna", name="dna")
                dnb = smp.tile([1, 512], F32, tag="dnb", name="dnb")
                nc.vector.tensor_copy(dna[:], pv0[64:65, :])
                nc.vector.tensor_copy(dnb[:], pv1[64:65, :])
                praw = otp.tile([128, 512], F32, tag="praw", name="praw")
                nc.vector.tensor_copy(praw[0:64, :], pv0[0:64, :])
                nc.vector.tensor_copy(praw[64:128, :], pv1[0:64, :])
                return dna, dnb, praw

            def epi_scale(dna, dnb, praw, smp, otp, psB):
                """Off-critical-path: recip, broadcast matmuls, fused scale.
                Returns pre-normalized outT fp16 (rows 0-63 h0, 64-127 h1)."""
                rca = smp.tile([1, 512], F32, tag="rca", name="rca")
                rcb = smp.tile([1, 512], F32, tag="rcb", name="rcb")
                nc.vector.reciprocal_approx_fast(rca[:], dna[:])
                nc.vector.reciprocal_approx_fast(rcb[:], dnb[:])
                rha = smp.tile([1, 512], F16, tag="rha", name="rha")
                rhb = smp.tile([1, 512], F16, tag="rhb", name="rhb")
                nc.vector.tensor_copy(rha[:], rca[:])
                nc.vector.tensor_copy(rhb[:], rcb[:])
                bc = psB.tile([128, 512], F32, tag="bc", name="bc")
                nc.tensor.matmul(bc[0:64, :], ones1[:], rha[:], start=True, stop=True)
                nc.tensor.matmul(
                    bc[64:128, :],
                    ones1[:],
                    rhb[:],
                    start=True,
                    stop=True,
                    tile_position=(0, 64),
                )
                outT = otp.tile([128, 512], F16, tag="outT", name="outT")
                nc.vector.tensor_tensor(outT[:], praw[:], bc[:], MUL)
                return outT

            def proj_qtile(qc, qs, outT, psP, obp):
                i = qc * 4 + qs
                pj = psP.tile([128, 512], F32, tag="pj", name="pj")
                nc.tensor.matmul(
                    pj[:],
                    outT[:, qs * 128 : (qs + 1) * 128],
                    wp_sb[:],
                    start=True,
                    stop=True,
                )
                ob = obp.tile([128, 512], F32, tag="ob", name="ob")
                nc.vector.tensor_copy(ob[:], pj[:])
                nc.sync.dma_start(out=out[i * 128 : (i + 1) * 128, :], in_=ob[:])

            with tc.tile_pool(name="ptp", bufs=5) as ptp, tc.tile_pool(
                name="smp", bufs=2
            ) as smp, tc.tile_pool(name="otp", bufs=2) as otp, tc.tile_pool(
                name="obp", bufs=2
            ) as obp, tc.tile_pool(
                name="psS", bufs=2, space="PSUM"
            ) as psS, tc.tile_pool(name="psV", bufs=1, space="PSUM") as psV:
                pv0 = psV.tile([65, 512], F32, tag="pv0", name="pv0")
                pv1 = psV.tile([65, 512], F32, tag="pv1", name="pv1")

                # ---- prologue: per 512-token slice produce kT/qT/v, with
                # qc=0's attention interleaved so ScalarE starts early
                with tc.tile_pool(name="psA", bufs=2, space="PSUM") as psA:
                    for s in range(n_s):
                        sl = slice(s * 512, (s + 1) * 512)
                        for kc in range(4):
                            nc.sync.dma_start(
                                out=xTs[kc][:, sl],
                                in_=xt[kc * 128 : (kc + 1) * 128, sl],
                            )
                        for w_sb, dst in ((wk_sb, kT), (wq_sb, qT)):
                            ps = psA.tile([128, 512], F32, tag="work", name="ps_kq")
                            for kc in range(4):
                                nc.tensor.matmul(
                                    ps[:],
                                    w_sb[:, kc * 128 : (kc + 1) * 128],
                                    xTs[kc][:, sl],
                                    start=(kc == 0),
                                    stop=(kc == 3),
                                )
                            nc.vector.tensor_copy(dst[s][:], ps[:])
                        # v natural: per 128-token block, accumulate over kc
                        vn = psA.tile([128, 512], F32, tag="work", name="vn")
                        for j in range(4):
                            tb = slice((4 * s + j) * 128, (4 * s + j + 1) * 128)
                            for kc in range(4):
                                nc.tensor.matmul(
                                    vn[:, j * 128 : (j + 1) * 128],
                                    xTs[kc][:, tb],
                                    wv_sb[:, kc * 128 : (kc + 1) * 128],
                                    start=(kc == 0),
                                    stop=(kc == 3),
                                )
                        for j in range(4):
                            t = 4 * s + j
                            nc.vector.tensor_copy(
                                v[t][:, 0:64], vn[:, j * 128 : j * 128 + 64]
                            )
                            nc.vector.tensor_copy(
                                v[t][:, 65:129], vn[:, j * 128 + 64 : (j + 1) * 128]
                            )
                            nc.gpsimd.memset(v[t][:, 64:65], 1.0)
                            nc.gpsimd.memset(v[t][:, 129:130], 1.0)
                        for kt in range(4 * s, 4 * s + 4):
                            pt = attn_S(0, kt, psS, ptp)
                            attn_PV(kt, pt, pv0, pv1)

                # ---- steady state: chunks 1..n_qc-1. Chunk qc-1's epilogue is
                # interleaved into chunk qc's kt loop: the first DEFER steps
                # emit only S+exp (PE stays busy) while the DVE drains the
                # previous chunk's PV accumulators; PV matmuls for those steps
                # are emitted after the drain so the in-order PE never blocks.
                DEFER = 4
                with tc.tile_pool(name="psB", bufs=1, space="PSUM") as psB, (
                    tc.tile_pool(name="psP", bufs=1, space="PSUM")
                ) as psP:
                    prev_pv = (pv0, pv1)
                    prev_qc = 0
                    outT = None
                    for qc in range(1, n_qc):
                        pv0 = psV.tile([65, 512], F32, tag="pv0", name="pv0")
                        pv1 = psV.tile([65, 512], F32, tag="pv1", name="pv1")
                        pts = [attn_S(qc, kt, psS, ptp) for kt in range(DEFER)]
                        drained = epi_drain(prev_pv[0], prev_pv[1], smp, otp)
                        for kt in range(DEFER):
                            attn_PV(kt, pts[kt], pv0, pv1)
                        for kt in range(DEFER, n_xt):
                            pt = attn_S(qc, kt, psS, ptp)
                            attn_PV(kt, pt, pv0, pv1)
                            if kt == DEFER + 2:
                                outT = epi_scale(*drained, smp, otp, psB)
                            if kt % 8 == 7:
                                proj_qtile(prev_qc, kt // 8, outT, psP, obp)
                        prev_pv = (pv0, pv1)
                        prev_qc = qc
                    drained = epi_drain(prev_pv[0], prev_pv[1], smp, otp)
                    outT = epi_scale(*drained, smp, otp, psB)
                    for qs in range(4):
                        proj_qtile(prev_qc, qs, outT, psP, obp)
    nc.compile()
    return nc


_CACHE = {}


def _get_nc(tokens=N):
    if tokens not in _CACHE:
        _CACHE[tokens] = build(tokens)
    return _CACHE[tokens]


def _prep_w(w_slice):
    """[512, 128] -> [128, 512] fp16 with w_[p, kc*128 + j] = w[kc*128 + p, j]."""
    w = np.asarray(w_slice, dtype=np.float32)
    return np.ascontiguousarray(
        w.reshape(4, 128, 128).transpose(1, 0, 2).reshape(128, 512).astype(np.float16)
    )


def _shard_inputs(x, w_qkv, w_proj):
    in_maps = []
    for c in range(8):
        b, hp = divmod(c, 4)
        o = 128 * hp
        in_maps.append(
            {
                "xt": np.ascontiguousarray(x[b].T.astype(np.float16)),
                "wq": _prep_w(w_qkv[:, o : o + 128]),
                "wk": _prep_w(w_qkv[:, 512 + o : 512 + o + 128]),
                "wv": _prep_w(w_qkv[:, 1024 + o : 1024 + o + 128]),
                "wp": np.ascontiguousarray(
                    w_proj[o : o + 128, :].astype(np.float16)
                ),
            }
        )
    return in_maps


def run(x, w_qkv, w_proj, b_proj, trace=False, **kwargs):
    from concourse.bass_utils import run_bass_kernel_spmd

    nc = _get_nc()
    in_maps = _shard_inputs(np.asarray(x), np.asarray(w_qkv), np.asarray(w_proj))
    br = run_bass_kernel_spmd(nc, in_maps, list(range(8)), trace=trace, **kwargs)
    parts = [np.asarray(br.results[c]["out"]) for c in range(8)]
    bp = np.asarray(b_proj)
    o0 = parts[0] + parts[1] + parts[2] + parts[3] + bp
    o1 = parts[4] + parts[5] + parts[6] + parts[7] + bp
    return np.stack([o0, o1]).astype(np.float32), br


def kernel(x, w_qkv, w_proj, b_proj):
    result, _ = run(x, w_qkv, w_proj, b_proj, trace=False)
    return result
